# revision 11
# baseline (speedup 1.0000x reference)
"""ALIGNNConv (2x EdgeGatedGraphConv) distributed Bass kernel, 8 TRN2 NeuronCores.

Self-contained: hardcodes shapes (F=128, N=50000, E=400000, LGE=800000) and the
sharding strategy; host preprocessing computes the shared SPMD tile schedule.

Per conv (i=dest, j=src):
  em   = x[i]@W0.T + x[j]@W1.T + e@W2.T + (b0+b1+b2)
  sig  = sigmoid(em);  u4 = x[j]@W4.T + b4
  SS_n = segsum_i(sig); PP_n = segsum_i(sig*u4); agg = PP/(SS+1e-9)
  e'   = e + silu(bn(em));  x' = x + silu(bn(x@W3.T + b3 + agg))

Distribution (both convs dest-sharded; all segment sums core-local):
  conv1: core c owns nodes [c*6272,(c+1)*6272); processes every edge whose dest
    is in its range, dest-sorted, slotted into a SHARED per-window tile schedule
    (T_w = max over cores of ceil(count/128)).  j-side rows come from a
    replicated node table U14=[x@W1.T | x@W4.T+b4] via per-tile
    indirect_dma_start.  i-side via S-matrix matmuls against U0=x@W0.T+b012
    windows.  Edge outputs (m = y + e_upd) live in "slot" coordinates.
  conv2: lg-node space = conv1 slot space; core c owns its own slots (the m
    rows it produced, SBUF-resident).  U14_2 built locally and AllGathered so
    j2 gathers hit a global table.  Outputs y_out/z_out in slot coords; host
    inverse-permutes.  BN stats via one tiny AllReduce per conv.
"""
import sys
import numpy as np

sys.path.insert(0, "/opt/trn_rl_repo")
import ml_dtypes

BF16 = ml_dtypes.bfloat16
P = 128
NCORES = 8


# =============================================================== host planning

def make_schedule(dst_by_core, n_windows):
    """dst_by_core: per-core sorted dest ids (device coords). Returns
    (schedule, slots_by_core): schedule = list over tiles of
    (window, first, last); slots_by_core[c] = per-tile list of edge index
    arrays (into the core's sorted order) with length <= 128."""
    counts = np.zeros((NCORES, n_windows), dtype=np.int64)
    for c in range(NCORES):
        w = dst_by_core[c] // P
        np.add.at(counts[c], w, 1)
    tw = np.maximum(1, np.ceil(counts.max(axis=0) / P).astype(np.int64))
    schedule = []
    for w in range(n_windows):
        for k in range(tw[w]):
            schedule.append((w, k == 0, k == tw[w] - 1))
    # per-core slot assignment
    starts = np.zeros((NCORES, n_windows + 1), dtype=np.int64)
    for c in range(NCORES):
        starts[c, 1:] = np.cumsum(counts[c])
    return schedule, tw, counts, starts


def pack_core(dst, jv, counts, starts, tw, schedule, nt):
    """Pack one core's sorted edges into the shared schedule.
    Returns S [P, nt*P], ST [P, nt*P], j_slots [nt*P], valid [nt*P],
    sel [nt*P] (index into core's sorted edge order, -1 for pads)."""
    S = np.zeros((P, nt * P), dtype=BF16)
    ST = np.zeros((P, nt * P), dtype=BF16)
    jsl = np.zeros(nt * P, dtype=np.int32)
    val = np.zeros(nt * P, dtype=BF16)
    sel = np.full(nt * P, -1, dtype=np.int64)
    t = 0
    for w in range(len(tw)):
        lo, hi = starts[w], starts[w + 1]
        for k in range(tw[w]):
            a = lo + k * P
            b = min(lo + (k + 1) * P, hi)
            if a < b:
                n = b - a
                rows = np.arange(n)
                cols = (dst[a:b] - w * P).astype(np.int64)
                S[rows, t * P + cols] = 1.0
                ST[cols, t * P + rows] = 1.0
                jsl[t * P:t * P + n] = jv[a:b]
                val[t * P:t * P + n] = 1.0
                sel[t * P:t * P + n] = np.arange(a, b)
            t += 1
    assert t == nt
    return S, ST, jsl, val, sel


def pmaj(v, nt, dtype):
    out = np.zeros((P, nt), dtype=dtype)
    idx = np.arange(len(v))
    out[idx % P, idx // P] = v
    return out


# =============================================================== bass builder

def build_kernel(cfg):
    import concourse.bass as bass
    import concourse.tile as tile
    from concourse import bacc, mybir
    from contextlib import ExitStack

    BF = mybir.dt.bfloat16
    F32 = mybir.dt.float32
    I32 = mybir.dt.int32
    AF = mybir.ActivationFunctionType

    NT1, NT2 = cfg["nt1"], cfg["nt2"]
    NN1 = cfg["nn1"]                 # conv1 padded global node count
    NW1 = NN1 // P
    NSLC = cfg["nslc"]               # conv1 node slice per core (NN1/8)
    NK1 = NSLC // P
    NSL = NT1 * P                    # conv2 node space per core (slot coords)
    NWL = NT1
    sched1 = cfg["sched1"]           # [(window, first, last)] len NT1
    sched2 = cfg["sched2"]           # len NT2 (windows in slot space)
    E1G, N1G, E2G, N2G = cfg["e1g"], cfg["n1g"], cfg["e2g"], cfg["n2g"]
    CH = 16
    SCH = 8
    ECH = 4

    nc = bacc.Bacc()

    def din(name, shape, dt=BF):
        return nc.declare_dram_parameter(name, shape, dt, isOutput=False)

    def dout(name, shape, dt=F32):
        return nc.declare_dram_parameter(name, shape, dt, isOutput=True)

    x_t = din("x_t", [P, NN1])
    x_sl_t = din("x_sl_t", [P, NSLC])
    y_t = din("y_t", [P, NT1 * P])
    z_t = din("z_t", [P, NT2 * P])
    S1i = din("S1", [P, NT1 * P]); ST1i = din("ST1", [P, NT1 * P])
    S2i = din("S2", [P, NT2 * P]); ST2i = din("ST2", [P, NT2 * P])
    j1i = din("j1", [P, NT1], I32); j2i = din("j2", [P, NT2], I32)
    va1 = din("va1", [P, NT1]); va2 = din("va2", [P, NT2])
    vn1 = din("vn1", [P, NK1]); vn2 = din("vn2", [P, NWL])
    wk1 = din("wk1", [P, 3 * P]); wk2 = din("wk2", [P, 3 * P])
    wb1 = din("wb1", [1, 3 * P]); wb2 = din("wb2", [1, 3 * P])
    w2c1 = din("w2c1", [P, P]); w2c2 = din("w2c2", [P, P])
    w3c1 = din("w3c1", [P, P]); w3c2 = din("w3c2", [P, P])
    w3b1 = din("w3b1", [1, P]); w3b2 = din("w3b2", [1, P])
    bnr = din("bnr", [1, 8 * P], F32)
    identi = din("identity", [P, P])

    x_out = dout("x_out", [P, NSLC])
    y_out = dout("y_out", [P, NSL])
    z_out = dout("z_out", [P, NT2 * P])

    u0_1 = nc.dram_tensor("u0_1", [NK1, P, P], BF)       # local x windows
    u14_1 = nc.dram_tensor("u14_1", [NN1, 2 * P], BF)    # global (replicated)
    u0_2 = nc.dram_tensor("u0_2", [NWL, P, P], BF)       # local m windows
    u14_2l = nc.dram_tensor("u14_2l", [NSL, 2 * P], BF)
    u14_2g = nc.dram_tensor("u14_2g", [NCORES * NSL, 2 * P], BF)
    em1s = nc.dram_tensor("em1s", [P, NT1 * P], BF)
    em2s = nc.dram_tensor("em2s", [P, NT2 * P], BF)
    agg1 = nc.dram_tensor("agg1", [NSLC, 2 * P], BF)
    agg2 = nc.dram_tensor("agg2", [NSL, 2 * P], BF)
    t1s = nc.dram_tensor("t1s", [NSLC, P], BF)
    t2s = nc.dram_tensor("t2s", [NSL, P], BF)

    with tile.TileContext(nc) as tc, ExitStack() as ctx:
        sb = ctx.enter_context(tc.tile_pool(name="sb", bufs=2))
        sbS = ctx.enter_context(tc.tile_pool(name="sbS", bufs=3))
        sbg = ctx.enter_context(tc.tile_pool(name="sbg", bufs=4))
        sbu = ctx.enter_context(tc.tile_pool(name="sbu", bufs=2))
        sbc = ctx.enter_context(tc.tile_pool(name="sbc", bufs=1))
        sbt = ctx.enter_context(tc.tile_pool(name="sbt", bufs=1))
        sbm = ctx.enter_context(tc.tile_pool(name="sbm", bufs=1))
        ps = ctx.enter_context(tc.tile_pool(name="ps", bufs=2, space="PSUM"))
        psw = ctx.enter_context(tc.tile_pool(name="psw", bufs=2, space="PSUM"))
        pss = ctx.enter_context(tc.tile_pool(name="pss", bufs=1, space="PSUM"))
        psm = ctx.enter_context(tc.tile_pool(name="psm", bufs=1, space="PSUM"))
        dr = ctx.enter_context(tc.tile_pool(name="dr", bufs=1, space="DRAM"))

        id_t = sbc.tile([P, P], BF)
        nc.sync.dma_start(out=id_t[:], in_=identi[:])
        wk1_t = sbc.tile([P, 3 * P], BF)
        nc.sync.dma_start(out=wk1_t[:], in_=wk1[:])
        wk2_t = sbc.tile([P, 3 * P], BF)
        nc.sync.dma_start(out=wk2_t[:], in_=wk2[:])
        wb1_t = sbc.tile([1, 3 * P], BF)
        nc.sync.dma_start(out=wb1_t[:], in_=wb1[:])
        wb2_t = sbc.tile([1, 3 * P], BF)
        nc.sync.dma_start(out=wb2_t[:], in_=wb2[:])
        w2c1_t = sbc.tile([P, P], BF)
        nc.sync.dma_start(out=w2c1_t[:], in_=w2c1[:])
        w2c2_t = sbc.tile([P, P], BF)
        nc.sync.dma_start(out=w2c2_t[:], in_=w2c2[:])
        w3c1_t = sbc.tile([P, P], BF)
        nc.sync.dma_start(out=w3c1_t[:], in_=w3c1[:])
        w3c2_t = sbc.tile([P, P], BF)
        nc.sync.dma_start(out=w3c2_t[:], in_=w3c2[:])
        w3b1_t = sbc.tile([1, P], BF)
        nc.sync.dma_start(out=w3b1_t[:], in_=w3b1[:])
        w3b2_t = sbc.tile([1, P], BF)
        nc.sync.dma_start(out=w3b2_t[:], in_=w3b2[:])
        bnr_t = sbc.tile([1, 8 * P], F32)
        nc.sync.dma_start(out=bnr_t[:], in_=bnr[:])
        ones_col = sbc.tile([P, 1], BF)
        nc.vector.memset(ones_col[:], 1.0)
        ones_row = sbc.tile([1, P], BF)
        nc.vector.memset(ones_row[:], 1.0)
        ones_f32 = sbc.tile([1, 1], F32)
        nc.vector.memset(ones_f32[:], 1.0)
        m_t = sbm.tile([P, NSL], BF)
        ab1 = sbc.tile([P, 4], F32)
        ab2 = sbc.tile([P, 4], F32)

        def build_u(src_ap, wk_t, wb_t, rlo, rhi, nwin, u0_dst, u14_dst):
            """Build U rows for node windows [0,nwin) of src (feature-major).
            rlo/rhi select which wk cols to compute: 0:P -> u0 only into
            u0_dst; P:3P -> u14 into u14_dst; 0:3P -> both."""
            wdt = rhi - rlo
            for k in range(nwin):
                up = psm.tile([P, 3 * P], F32, space="PSUM", tag="pm")
                xt = sb.tile([P, P], BF, tag="ubx")
                nc.sync.dma_start(out=xt[:], in_=src_ap[:, k * P:(k + 1) * P])
                nc.tensor.matmul(out=up[:, 0:wdt], lhsT=xt[:],
                                 rhs=wk_t[:, rlo:rhi], start=True, stop=False,
                                 skip_group_check=True)
                nc.tensor.matmul(out=up[:, 0:wdt], lhsT=ones_row[:],
                                 rhs=wb_t[:, rlo:rhi], start=False,
                                 stop=True, skip_group_check=True)
                ub = sb.tile([P, 3 * P], BF, tag="ubo")
                nc.scalar.copy(out=ub[:, 0:wdt], in_=up[:, 0:wdt])
                if rlo == 0:
                    nc.scalar.dma_start(out=u0_dst[k], in_=ub[:, 0:P])
                if rhi == 3 * P:
                    off = P - rlo
                    nc.scalar.dma_start(out=u14_dst[k * P:(k + 1) * P, :],
                                        in_=ub[:, off:off + 2 * P])

        def edge_pass_a(nt, sched, jx, vax, Sx, STx, ytab, w2t, u0tab, u14tab,
                        emscr, aggdst, estat):
            wps = [None]
            u0w = [None]
            for t in range(nt):
                w, first, last = sched[t]
                ci = t % CH
                if ci == 0:
                    cw = min(CH, nt - t)
                    jt = sbg.tile([P, CH], I32, tag="jt")
                    nc.scalar.dma_start(out=jt[:, 0:cw], in_=jx[:, t:t + cw])
                    vat = sbg.tile([P, CH], BF, tag="vat")
                    nc.scalar.dma_start(out=vat[:, 0:cw], in_=vax[:, t:t + cw])
                si = t % SCH
                if si == 0:
                    sw = min(SCH, nt - t) * P
                    s_c = sbS.tile([P, SCH * P], BF, tag="s")
                    nc.sync.dma_start(out=s_c[:, 0:sw], in_=Sx[:, t * P:t * P + sw])
                    st_c = sbS.tile([P, SCH * P], BF, tag="st")
                    nc.sync.dma_start(out=st_c[:, 0:sw], in_=STx[:, t * P:t * P + sw])
                    y_c = sbS.tile([P, SCH * P], BF, tag="y")
                    nc.sync.dma_start(out=y_c[:, 0:sw], in_=ytab[:, t * P:t * P + sw])
                ei = t % ECH
                if ei == 0:
                    eq4 = sbg.tile([P, ECH * 2 * P], BF, tag="eq4")
                if first:
                    u0wt = sbu.tile([P, P], BF, tag="u0w")
                    nc.sync.dma_start(out=u0wt[:], in_=u0tab[w])
                    u0w[0] = u0wt
                g = sbg.tile([P, 2 * P], BF, tag="g")
                nc.gpsimd.indirect_dma_start(
                    out=g[:], out_offset=None, in_=u14tab[:],
                    in_offset=bass.IndirectOffsetOnAxis(ap=jt[:, ci:ci + 1], axis=0))
                emp = ps.tile([P, 2 * P], F32, space="PSUM", tag="emp")
                nc.tensor.matmul(out=emp[:, P:2 * P], lhsT=id_t[:],
                                 rhs=g[:, P:2 * P], start=True, stop=True,
                                 skip_group_check=True)
                nc.tensor.matmul(out=emp[:, 0:P], lhsT=y_c[:, si * P:(si + 1) * P],
                                 rhs=w2t[:], start=True, stop=False,
                                 skip_group_check=True)
                nc.tensor.matmul(out=emp[:, 0:P], lhsT=id_t[:], rhs=g[:, 0:P],
                                 start=False, stop=False, skip_group_check=True)
                nc.tensor.matmul(out=emp[:, 0:P],
                                 lhsT=st_c[:, si * P:(si + 1) * P],
                                 rhs=u0w[0][:], start=False, stop=True,
                                 skip_group_check=True)
                spc = sb.tile([P, 2 * P], BF, tag="spc")
                nc.scalar.activation(out=spc[:, 0:P], in_=emp[:, 0:P],
                                     func=AF.Sigmoid)
                nc.scalar.copy(out=eq4[:, ei * 2 * P:ei * 2 * P + P],
                               in_=emp[:, 0:P])
                nc.scalar.square(out=eq4[:, ei * 2 * P + P:(ei + 1) * 2 * P],
                                 in_=emp[:, 0:P])
                nc.vector.tensor_mul(out=spc[:, P:2 * P], in0=spc[:, 0:P],
                                     in1=emp[:, P:2 * P])
                if ei == ECH - 1 or t == nt - 1:
                    base = (t - ei) * P
                    nc.sync.dma_start(
                        out=emscr[:, base:base + (ei + 1) * P],
                        in_=eq4[:].rearrange("p (e c) -> p e c", c=2 * P)[:, :ei + 1, 0:P])
                if first:
                    wpst = psw.tile([P, 2 * P], F32, space="PSUM", tag="wps")
                    wps[0] = wpst
                nc.tensor.matmul(out=wps[0][:], lhsT=s_c[:, si * P:(si + 1) * P],
                                 rhs=spc[:], start=first, stop=last,
                                 skip_group_check=True)
                nc.tensor.matmul(out=estat[:], lhsT=vat[:, ci:ci + 1],
                                 rhs=eq4[:, ei * 2 * P:(ei + 1) * 2 * P],
                                 start=(t == 0), stop=(t == nt - 1),
                                 skip_group_check=True)
                if last:
                    wt = sb.tile([P, 2 * P], BF, tag="wout")
                    nc.scalar.copy(out=wt[:], in_=wps[0][:])
                    nc.scalar.dma_start(out=aggdst[w * P:(w + 1) * P, :], in_=wt[:])

        def node_pass(nk, xsrc_ap, w3t, w3b, aggsrc, vnx, tscr, nstat):
            for k in range(nk):
                tp = psm.tile([P, 3 * P], F32, space="PSUM", tag="pm")
                xt = sb.tile([P, P], BF, tag="nx")
                nc.sync.dma_start(out=xt[:], in_=xsrc_ap[:, k * P:(k + 1) * P])
                nc.tensor.matmul(out=tp[:, 0:P], lhsT=xt[:], rhs=w3t[:],
                                 start=True, stop=False, skip_group_check=True)
                nc.tensor.matmul(out=tp[:, 0:P], lhsT=ones_row[:], rhs=w3b[:],
                                 start=False, stop=False, skip_group_check=True)
                at = sb.tile([P, 2 * P], BF, tag="nagg")
                nc.sync.dma_start(out=at[:], in_=aggsrc[k * P:(k + 1) * P, :])
                ssf = sb.tile([P, P], F32, tag="nssf")
                nc.vector.tensor_scalar_add(out=ssf[:], in0=at[:, 0:P],
                                            scalar1=1e-9)
                rcp = sb.tile([P, P], F32, tag="nrcp")
                nc.vector.reciprocal(out=rcp[:], in_=ssf[:])
                ppf = sb.tile([P, P], F32, tag="nppf")
                nc.scalar.copy(out=ppf[:], in_=at[:, P:2 * P])
                agt = sb.tile([P, P], BF, tag="nagt")
                nc.vector.tensor_mul(out=agt[:], in0=ppf[:], in1=rcp[:])
                nc.tensor.matmul(out=tp[:, 0:P], lhsT=id_t[:], rhs=agt[:],
                                 start=False, stop=True, skip_group_check=True)
                tq = sb.tile([P, 2 * P], BF, tag="ntq")
                nc.scalar.copy(out=tq[:, 0:P], in_=tp[:, 0:P])
                nc.scalar.square(out=tq[:, P:2 * P], in_=tp[:, 0:P])
                vt = sb.tile([P, 1], BF, tag="nvt")
                nc.sync.dma_start(out=vt[:], in_=vnx[:, k:k + 1])
                nc.tensor.matmul(out=nstat[:], lhsT=vt[:], rhs=tq[:],
                                 start=(k == 0), stop=(k == nk - 1),
                                 skip_group_check=True)
                nc.scalar.dma_start(out=tscr[k * P:(k + 1) * P, :], in_=tq[:, 0:P])

        def stats_to_ab(estat, nstat, ecount, ncount, goff, ab_t):
            srow = sbt.tile([1, 512], F32, tag="srow")
            nc.scalar.copy(out=srow[:, 0:256], in_=estat[:])
            nc.scalar.copy(out=srow[:, 256:512], in_=nstat[:])
            cin = dr.tile([1, 512], F32)
            cout = dr.tile([1, 512], F32)
            nc.sync.dma_start(out=cin[:], in_=srow[:])
            from concourse import mybir as mb
            nc.gpsimd.collective_compute(
                "AllReduce", mb.AluOpType.add,
                replica_groups=[list(range(NCORES))],
                ins=[cin.opt()], outs=[cout.opt()])
            gr = sbt.tile([1, 512], F32, tag="gr")
            nc.sync.dma_start(out=gr[:], in_=cout[:])
            abrow = sbt.tile([1, 512], F32, tag="abrow")
            for gi, (off, cnt) in enumerate(((0, ecount), (256, ncount))):
                mean = sbt.tile([1, P], F32, tag="stm")
                nc.scalar.mul(out=mean[:], in_=gr[:, off:off + P], mul=1.0 / cnt)
                msq = sbt.tile([1, P], F32, tag="stq")
                nc.scalar.mul(out=msq[:], in_=gr[:, off + P:off + 2 * P],
                              mul=1.0 / cnt)
                mm2 = sbt.tile([1, P], F32, tag="stm2")
                nc.vector.tensor_mul(out=mm2[:], in0=mean[:], in1=mean[:])
                var = sbt.tile([1, P], F32, tag="stv")
                nc.vector.tensor_sub(out=var[:], in0=msq[:], in1=mm2[:])
                vpe = sbt.tile([1, P], F32, tag="stvpe")
                nc.vector.tensor_scalar_add(out=vpe[:], in0=var[:], scalar1=1e-5)
                sd = sbt.tile([1, P], F32, tag="stsd")
                nc.scalar.activation(out=sd[:], in_=vpe[:], func=AF.Sqrt)
                rsd = sbt.tile([1, P], F32, tag="strsd")
                nc.vector.reciprocal(out=rsd[:], in_=sd[:])
                A = sbt.tile([1, P], F32, tag="stA")
                ga = (goff + 2 * gi) * P
                nc.vector.tensor_mul(out=A[:], in0=bnr_t[:, ga:ga + P],
                                     in1=rsd[:])
                mA = sbt.tile([1, P], F32, tag="stmA")
                nc.vector.tensor_mul(out=mA[:], in0=mean[:], in1=A[:])
                B = sbt.tile([1, P], F32, tag="stB")
                nc.vector.tensor_sub(out=B[:], in0=bnr_t[:, ga + P:ga + 2 * P],
                                     in1=mA[:])
                nc.scalar.copy(out=abrow[:, gi * 256:gi * 256 + P], in_=A[:])
                nc.scalar.copy(out=abrow[:, gi * 256 + P:gi * 256 + 2 * P],
                               in_=B[:])
            for col in range(4):
                tp = psm.tile([P, 3 * P], F32, space="PSUM", tag="pm")
                nc.tensor.matmul(out=tp[:, 0:1], lhsT=abrow[:, col * P:(col + 1) * P],
                                 rhs=ones_f32[:], start=True, stop=True,
                                 skip_group_check=True)
                nc.scalar.copy(out=ab_t[:, col:col + 1], in_=tp[:, 0:1])

        def edge_pass_b(nt, emscr, etab, ab_t, out_ft, to_dram):
            for t in range(nt):
                si = t % SCH
                if si == 0:
                    sw = min(SCH, nt - t) * P
                    e_c = sbS.tile([P, SCH * P], BF, tag="pby")
                    nc.sync.dma_start(out=e_c[:, 0:sw], in_=etab[:, t * P:t * P + sw])
                emt = sb.tile([P, P], BF, tag="pbe")
                nc.sync.dma_start(out=emt[:], in_=emscr[:, t * P:(t + 1) * P])
                tp = psm.tile([P, 3 * P], BF, space="PSUM", tag="pmt")
                nc.tensor.transpose(out=tp[:, 0:P], in_=emt[:], identity=id_t[:])
                if not to_dram:
                    eu = sb.tile([P, P], BF, tag="pbu")
                    nc.scalar.activation(out=eu[:], in_=tp[:, 0:P], func=AF.Silu,
                                         bias=ab_t[:, 1:2], scale=ab_t[:, 0:1])
                    nc.vector.tensor_add(out=out_ft[:, t * P:(t + 1) * P],
                                         in0=e_c[:, si * P:(si + 1) * P],
                                         in1=eu[:])
                else:
                    eu = sb.tile([P, P], F32, tag="pbuf")
                    nc.scalar.activation(out=eu[:], in_=tp[:, 0:P], func=AF.Silu,
                                         bias=ab_t[:, 1:2], scale=ab_t[:, 0:1])
                    zo = sb.tile([P, P], F32, tag="pbz")
                    nc.vector.tensor_add(out=zo[:],
                                         in0=e_c[:, si * P:(si + 1) * P],
                                         in1=eu[:])
                    nc.scalar.dma_start(out=out_ft[:, t * P:(t + 1) * P],
                                        in_=zo[:])

        def node_pass_b(nk, tscr, xsrc_ap, ab_t, xout, src_sbuf):
            for k in range(nk):
                tt = sb.tile([P, P], BF, tag="nbt")
                nc.sync.dma_start(out=tt[:], in_=tscr[k * P:(k + 1) * P, :])
                tp = psm.tile([P, 3 * P], BF, space="PSUM", tag="pmt")
                nc.tensor.transpose(out=tp[:, 0:P], in_=tt[:], identity=id_t[:])
                xu = sb.tile([P, P], F32, tag="nbu")
                nc.scalar.activation(out=xu[:], in_=tp[:, 0:P], func=AF.Silu,
                                     bias=ab_t[:, 3:4], scale=ab_t[:, 2:3])
                if src_sbuf:
                    xsl = xsrc_ap[:, k * P:(k + 1) * P]
                else:
                    xst = sb.tile([P, P], BF, tag="nbs")
                    nc.sync.dma_start(out=xst[:], in_=xsrc_ap[:, k * P:(k + 1) * P])
                    xsl = xst[:]
                xf = sb.tile([P, P], F32, tag="nbx")
                nc.scalar.copy(out=xf[:], in_=xsl)
                xo = sb.tile([P, P], F32, tag="nbo")
                nc.vector.tensor_add(out=xo[:], in0=xf[:], in1=xu[:])
                nc.scalar.dma_start(out=xout[:, k * P:(k + 1) * P], in_=xo[:])

        # ================= conv1 =================
        estat1 = pss.tile([1, 2 * P], F32, space="PSUM", tag="es")
        nstat1 = pss.tile([1, 2 * P], F32, space="PSUM", tag="ns")
        build_u(x_t[:], wk1_t, wb1_t, P, 3 * P, NW1, u0_1, u14_1)
        build_u(x_sl_t[:], wk1_t, wb1_t, 0, P, NK1, u0_1, u14_1)
        edge_pass_a(NT1, sched1, j1i, va1, S1i, ST1i, y_t, w2c1_t[:],
                    u0_1, u14_1, em1s, agg1, estat1)
        node_pass(NK1, x_sl_t[:], w3c1_t[:], w3b1_t[:], agg1, vn1, t1s, nstat1)
        stats_to_ab(estat1, nstat1, E1G, N1G, 0, ab1)
        edge_pass_b(NT1, em1s, y_t, ab1, m_t, False)
        node_pass_b(NK1, t1s, x_sl_t[:], ab1, x_out, False)

        # ================= conv2 =================
        build_u(m_t[:], wk2_t, wb2_t, 0, 3 * P, NWL, u0_2, u14_2l)
        agi = dr.tile([NSL, 2 * P], BF)
        ago = dr.tile([NCORES * NSL, 2 * P], BF)
        nc.gpsimd.dma_start(out=agi[:], in_=u14_2l[:])
        from concourse import mybir as mb
        nc.gpsimd.collective_compute(
            "AllGather", mb.AluOpType.bypass,
            replica_groups=[list(range(NCORES))],
            ins=[agi.opt()], outs=[ago.opt()])
        nc.gpsimd.dma_start(out=u14_2g[:], in_=ago[:])
        estat2 = pss.tile([1, 2 * P], F32, space="PSUM", tag="es")
        nstat2 = pss.tile([1, 2 * P], F32, space="PSUM", tag="ns")
        edge_pass_a(NT2, sched2, j2i, va2, S2i, ST2i, z_t, w2c2_t[:],
                    u0_2, u14_2g, em2s, agg2, estat2)
        node_pass(NWL, m_t[:], w3c2_t[:], w3b2_t[:], agg2, vn2, t2s, nstat2)
        stats_to_ab(estat2, nstat2, E2G, N2G, 4, ab2)
        edge_pass_b(NT2, em2s, z_t, ab2, z_out, True)
        node_pass_b(NWL, t2s, m_t[:], ab2, y_out, True)

    nc.compile()
    return nc


# =============================================================== host driver

def _prepare(inputs):
    gi = np.asarray(inputs["g_edge_index"], dtype=np.int64)
    li = np.asarray(inputs["lg_edge_index"], dtype=np.int64)
    x = np.asarray(inputs["x"], dtype=np.float32)
    y = np.asarray(inputs["y"], dtype=np.float32)
    z = np.asarray(inputs["z"], dtype=np.float32)
    n_nodes, n_edges, n_lg = x.shape[0], y.shape[0], z.shape[0]

    nn1 = ((n_nodes + 1023) // 1024) * 1024
    nslc = nn1 // NCORES

    # ---- conv1: dest-sharded
    own1 = np.minimum(gi[0] // nslc, NCORES - 1)
    perm1, dst1, j1 = [], [], []
    for c in range(NCORES):
        sel = np.where(own1 == c)[0]
        order = sel[np.argsort(gi[0][sel], kind="stable")]
        perm1.append(order)
        dst1.append(gi[0][order] - c * nslc)
        j1.append(gi[1][order])
    sched1_raw, tw1, cnt1, starts1 = make_schedule(dst1, nslc // P)
    nt1 = len(sched1_raw)

    # slot of each original edge in its owner's schedule
    slot1 = np.zeros(n_edges, dtype=np.int64)
    packs1 = []
    for c in range(NCORES):
        S, ST, jsl, val, sel = pack_core(dst1[c], j1[c], cnt1[c], starts1[c],
                                         tw1, sched1_raw, nt1)
        packs1.append((S, ST, jsl, val, sel))
        slots = np.where(sel >= 0)[0]
        slot1[perm1[c][sel[slots]]] = slots
    nsl = nt1 * P

    # ---- conv2: lg-node space = conv1 slot space
    e_owner = own1                       # owner core of g-edge (lg-node)
    d2dev = slot1[li[0]]                 # device dest id (slot on owner core)
    own2 = e_owner[li[0]]
    jglob = e_owner[li[1]] * nsl + slot1[li[1]]
    perm2, dst2, j2 = [], [], []
    for c in range(NCORES):
        sel = np.where(own2 == c)[0]
        order = sel[np.argsort(d2dev[sel], kind="stable")]
        perm2.append(order)
        dst2.append(d2dev[order])
        j2.append(jglob[order])
    sched2_raw, tw2, cnt2, starts2 = make_schedule(dst2, nsl // P)
    nt2 = len(sched2_raw)
    packs2 = [pack_core(dst2[c], j2[c], cnt2[c], starts2[c], tw2,
                        sched2_raw, nt2) for c in range(NCORES)]

    W1p, b1p = np.asarray(inputs["W1"], np.float32), np.asarray(inputs["b1"], np.float32)
    W2p, b2p = np.asarray(inputs["W2"], np.float32), np.asarray(inputs["b2"], np.float32)
    bn_g1, bn_b1 = np.asarray(inputs["bn_g1"], np.float32), np.asarray(inputs["bn_b1"], np.float32)
    bn_g2, bn_b2 = np.asarray(inputs["bn_g2"], np.float32), np.asarray(inputs["bn_b2"], np.float32)

    def wkpack(W, b):
        wk = np.zeros((P, 3 * P), dtype=BF16)
        wk[:, 0:P] = W[0].T.astype(BF16)
        wk[:, P:2 * P] = W[1].T.astype(BF16)
        wk[:, 2 * P:3 * P] = W[4].T.astype(BF16)
        return wk

    def wbpack(b):
        wb = np.zeros((1, 3 * P), dtype=BF16)
        wb[0, 0:P] = (b[0] + b[1] + b[2]).astype(BF16)
        wb[0, 2 * P:3 * P] = b[4].astype(BF16)
        return wb

    bnrm = np.stack([bn_g1[0], bn_b1[0], bn_g1[1], bn_b1[1],
                     bn_g2[0], bn_b2[0], bn_g2[1], bn_b2[1]]).astype(np.float32)
    ident = np.eye(P, dtype=BF16)
    x_t_full = np.zeros((P, nn1), dtype=BF16)
    x_t_full[:, :n_nodes] = x.T.astype(BF16)

    def slot_feat(rows, sel, nt):
        out = np.zeros((P, nt * P), dtype=BF16)
        ok = np.where(sel >= 0)[0]
        out[:, ok] = rows[sel[ok]].astype(BF16).T
        return out

    core_maps = []
    for c in range(NCORES):
        S1, ST1, j1s, va1v, sel1 = packs1[c]
        S2, ST2, j2s, va2v, sel2 = packs2[c]
        lo = c * nslc
        vn1v = ((np.arange(nslc) + lo) < n_nodes).astype(BF16)
        vn2v = np.zeros(nsl, dtype=BF16)
        vn2v[np.where(sel1 >= 0)[0]] = 1.0
        core_maps.append({
            "x_t": x_t_full,
            "x_sl_t": np.ascontiguousarray(x_t_full[:, lo:lo + nslc]),
            "y_t": slot_feat(y[perm1[c]], sel1, nt1),
            "z_t": slot_feat(z[perm2[c]], sel2, nt2),
            "S1": S1, "ST1": ST1, "S2": S2, "ST2": ST2,
            "j1": pmaj(j1s, nt1, np.int32), "j2": pmaj(j2s, nt2, np.int32),
            "va1": pmaj(va1v, nt1, BF16), "va2": pmaj(va2v, nt2, BF16),
            "vn1": pmaj(vn1v, nslc // P, BF16), "vn2": pmaj(vn2v, nsl // P, BF16),
            "wk1": wkpack(W1p, b1p), "wk2": wkpack(W2p, b2p),
            "wb1": wbpack(b1p), "wb2": wbpack(b2p),
            "w2c1": W1p[2].T.astype(BF16), "w2c2": W2p[2].T.astype(BF16),
            "w3c1": W1p[3].T.astype(BF16), "w3c2": W2p[3].T.astype(BF16),
            "w3b1": b1p[3].reshape(1, P).astype(BF16),
            "w3b2": b2p[3].reshape(1, P).astype(BF16),
            "bnr": bnrm.reshape(1, -1), "identity": ident,
        })

    cfg = {"nt1": nt1, "nt2": nt2, "nn1": nn1, "nslc": nslc,
           "sched1": sched1_raw, "sched2": sched2_raw,
           "e1g": float(n_edges), "n1g": float(n_nodes),
           "e2g": float(n_lg), "n2g": float(n_edges)}
    meta = {"perm1": perm1, "perm2": perm2, "packs1": packs1, "packs2": packs2,
            "nslc": nslc, "nsl": nsl, "nt2": nt2,
            "n_nodes": n_nodes, "n_edges": n_edges, "n_lg": n_lg}
    return core_maps, cfg, meta


def _assemble(results, meta):
    n_nodes, n_edges, n_lg = meta["n_nodes"], meta["n_edges"], meta["n_lg"]
    nslc = meta["nslc"]
    x_out = np.zeros((n_nodes, P), dtype=np.float32)
    y_out = np.zeros((n_edges, P), dtype=np.float32)
    z_out = np.zeros((n_lg, P), dtype=np.float32)
    for c in range(NCORES):
        r = results[c]
        lo = c * nslc
        hi = min(lo + nslc, n_nodes)
        if hi > lo:
            x_out[lo:hi] = r["x_out"].T[: hi - lo]
        sel1 = meta["packs1"][c][4]
        ok1 = np.where(sel1 >= 0)[0]
        y_out[meta["perm1"][c][sel1[ok1]]] = r["y_out"].T[ok1]
        sel2 = meta["packs2"][c][4]
        ok2 = np.where(sel2 >= 0)[0]
        z_out[meta["perm2"][c][sel2[ok2]]] = r["z_out"].T[ok2]
    return x_out, y_out, z_out


_CACHE = {}


def kernel(**inputs):
    from concourse.bass_utils import run_bass_kernel_spmd
    core_maps, cfg, meta = _prepare(inputs)
    key = (cfg["nt1"], cfg["nt2"], cfg["nn1"],
           tuple(w for w, _, _ in cfg["sched1"]),
           tuple(w for w, _, _ in cfg["sched2"]))
    if key not in _CACHE:
        _CACHE[key] = build_kernel(cfg)
    nc = _CACHE[key]
    res = run_bass_kernel_spmd(nc, core_maps, list(range(NCORES)))
    return _assemble(res.results, meta)
